# revision 20
# baseline (speedup 1.0000x reference)
"""GCN mean-aggregation + linear on 8 Trainium2 NeuronCores.

out = (segment_sum(x[col], row) / deg(row)) @ W.T + b

Strategy:
  - Destinations (rows of the output) are sharded across 8 cores, 6250 each.
  - Sources are split into lo/hi halves of 25000 rows so gather indices fit
    int16 (dma_gather sign-extends its indices).
  - x is cast to bf16 on the host; each core bulk-gathers the 256-byte
    feature rows of its edges with dma_gather (edge-on-partition layout).
  - Segment-sum is done on the TensorEngine: for each 128-edge tile,
    psum[feat, c] += Y.T @ S where S is a streamed [128, 32] matrix that
    maps each edge slot to its destination column, with value 1/deg(dst)
    (the mean division folded in).  Tile t owns the 16 destinations
    [16t, 16t+16) and can also address the next 16 (window 32), which lets
    an overflowing tile push excess edges one tile backward.
  - Edges that still don't fit (~2.4%) are fixed up on the host.
  - Rounds iterate over destination windows with BOTH source halves
    accumulating into one PSUM region.  Each round's gather work is split
    into 4 pieces (half-lo a/b, half-hi a/b) pinned to SWDGE queues 0-3 so
    all four queue ucode streams generate descriptors concurrently (the
    old 2-gathers-per-round layout left half the queues idle each round).
  - idx is loaded as ONE whole-tensor DMA (12.5KB-per-partition
    descriptors); smat streams per-round in 2-4KB-per-partition chunks.
    The old 152 small column-sliced DMAs held the sync sequencer ~50%
    busy and starved the gathers via $S-wait head-of-line blocking.
  - dynamic_dma_scratch_size is raised to 64KB: the decode-side
    await_space blocks the whole GPSIMD instruction stream when the next
    gather's descriptors don't fit in the ring until older DMAs complete,
    so ring capacity is effectively the gather pipeline depth.
  - The final 128x128 linear runs per-round during the main loop
    (W.T stationary, aggT moving), leaving almost no serial tail.
"""

import os
import sys

import numpy as np
import ml_dtypes

# ---- static problem configuration (hardcoded per the task contract) ----
N_NODES = 50000
N_EDGES = 800000
D = 128
NC = 8
SHARD = N_NODES // NC          # 6250 destinations per core
HALF = N_NODES // 2            # 25000 source rows per gather table
V_T = 16                       # destinations owned per 128-slot tile
M_WIN = 32                     # S window columns (own 16 + next 16)
TILES_HALF = -(-SHARD // V_T)  # 391
N_TILES = 2 * TILES_HALF       # 782
PSUM_BANK = 512                # f32 columns per PSUM bank

# super-round schedule: gathers are issued at super-round granularity
# (4 instructions on queues 0-3, up to 4096 idx each) to amortize the
# ~1.5us fixed cost per gather ucode instruction; PSUM/evict/final run in
# 32-tile sub-rounds within each super-round.  Small head super-rounds
# get all 4 queues generating within ~1us.
SROUND_W = [8, 16] + [64] * 5 + [47]
assert sum(SROUND_W) == TILES_HALF

for _p in ("/opt/trn_rl_repo", "/root/.axon_site/_ro/trn_rl_repo"):
    if os.path.isdir(_p) and _p not in sys.path:
        sys.path.insert(0, _p)


# --------------------------------------------------------------------------
# Host-side edge packing
# --------------------------------------------------------------------------

def _pack_half(vd, src, shard, v_t, m_win, tiles_half):
    """Pack one (core, half)'s edges into fixed 128-slot tiles.

    vd:  local destination id in [0, shard) per edge
    src: gather index in [0, HALF) per edge

    Returns (slot_idx[tiles*128] int16, slot_col[tiles*128] int8 (-1 = pad),
             spill_mask over input edges).
    """
    n = len(vd)
    order = np.argsort(vd, kind="stable")
    vd_s = vd[order]
    src_s = src[order]
    cnt = np.bincount(vd_s, minlength=shard)
    # per-tile own edge ranges in the sorted arrays
    tile_cnt = np.add.reduceat(cnt, np.arange(0, shard, v_t)) if shard else np.zeros(0, np.int64)
    tile_end = np.cumsum(tile_cnt)
    tile_start = tile_end - tile_cnt

    slot_idx = np.zeros(tiles_half * 128, dtype=np.int16)
    slot_col = np.full(tiles_half * 128, -1, dtype=np.int8)
    spill_sorted = np.zeros(n, dtype=bool)

    kept = np.minimum(tile_cnt, 128)
    moved = np.zeros(tiles_half, dtype=np.int64)   # edges moved from t into t-1
    spill_ct = np.zeros(tiles_half, dtype=np.int64)
    for t in range(tiles_half):
        excess = tile_cnt[t] - 128
        if excess > 0:
            if t > 0:
                # tile t-1 receives edges only from t, so its free space
                # when t is processed is just 128 - kept[t-1]
                moved[t] = min(excess, max(0, 128 - kept[t - 1]))
            spill_ct[t] = excess - moved[t]

    for t in range(tiles_half):
        s, e = tile_start[t], tile_end[t]
        own_keep = kept[t]  # first own_keep own edges stay
        # own kept edges -> slots [0, own_keep)
        sl = t * 128
        if own_keep > 0:
            sel = slice(s, s + own_keep)
            slot_idx[sl:sl + own_keep] = src_s[sel]
            slot_col[sl:sl + own_keep] = (vd_s[sel] - t * v_t).astype(np.int8)
        pos = own_keep
        # moved-in edges from tile t+1
        if t + 1 < tiles_half and moved[t + 1] > 0:
            m = moved[t + 1]
            s2 = tile_start[t + 1] + kept[t + 1]
            sel = slice(s2, s2 + m)
            slot_idx[sl + pos:sl + pos + m] = src_s[sel]
            cols = vd_s[sel] - t * v_t
            assert cols.min() >= 0 and cols.max() < m_win
            slot_col[sl + pos:sl + pos + m] = cols.astype(np.int8)
            pos += m
        # spilled edges of tile t -> host
        if spill_ct[t] > 0:
            s3 = tile_start[t] + kept[t] + moved[t]
            spill_sorted[s3:s3 + spill_ct[t]] = True

    spill_mask = np.zeros(n, dtype=bool)
    spill_mask[order] = spill_sorted
    return slot_idx, slot_col, spill_mask


def _pack_inputs(x, edge_index, W, b):
    """Build per-core numpy input dicts + host spill info."""
    x = np.asarray(x, dtype=np.float32)
    ei = np.asarray(edge_index)
    W = np.asarray(W, dtype=np.float32)
    b = np.asarray(b, dtype=np.float32)
    row = ei[0].astype(np.int64)
    col = ei[1].astype(np.int64)

    x_bf = x.astype(ml_dtypes.bfloat16)
    xlo = np.ascontiguousarray(x_bf[:HALF])
    xhi = np.ascontiguousarray(x_bf[HALF:])
    wt = np.ascontiguousarray(W.T)                       # [din, dout]
    bT = np.ascontiguousarray(b.reshape(128, 1))

    deg = np.bincount(row, minlength=N_NODES).astype(np.float64)
    with np.errstate(divide="ignore"):
        recip_full = np.where(deg > 0, 1.0 / deg, 0.0).astype(np.float32)

    core = row // SHARD
    in_maps = []
    spill_rows = []
    for k in range(NC):
        sel = core == k
        r = row[sel] - k * SHARD
        c = col[sel]
        lo = c < HALF

        idx_all = np.zeros(N_TILES * 128, dtype=np.int16)
        col_all = np.full(N_TILES * 128, -1, dtype=np.int8)
        for h in (0, 1):
            hm = lo if h == 0 else ~lo
            vd = r[hm]
            src = (c[hm] - h * HALF).astype(np.int64)
            si, sc, sp = _pack_half(vd, src, SHARD, V_T, M_WIN, TILES_HALF)
            o = h * TILES_HALF * 128
            idx_all[o:o + TILES_HALF * 128] = si
            col_all[o:o + TILES_HALF * 128] = sc
            if sp.any():
                spill_rows.append(np.nonzero(sel)[0][hm][sp])

        # compact S descriptors: per slot (p, t) its window column (-1 pad)
        # and 1/deg(dst); the dense [128, M_WIN]-per-tile S matrix is
        # generated on device (iota==col mask * recip), killing the 6.4MB
        # smat stream
        colv = np.ascontiguousarray(
            col_all.reshape(N_TILES, 128).T.astype(ml_dtypes.bfloat16))
        recipv = np.zeros((128, N_TILES), dtype=ml_dtypes.bfloat16)
        slots = np.nonzero(col_all >= 0)[0]
        t_of = slots // 128
        p_of = slots % 128
        dst_local = (t_of % TILES_HALF) * V_T + col_all[slots]
        recipv[p_of, t_of] = recip_full[k * SHARD + dst_local]
        iot = np.ascontiguousarray(
            np.tile(np.arange(M_WIN), (128, 1)).astype(ml_dtypes.bfloat16))

        # gather index layout: global slot i -> [i % 16, i // 16], and the
        # 16-partition pattern replicated to all 128 partitions (the Q7
        # desc-gen cores each read their own 16-partition stripe)
        idx16 = np.ascontiguousarray(
            np.tile(idx_all.reshape(-1, 16).T, (8, 1)))

        in_maps.append({
            "xlo": xlo, "xhi": xhi,
            "idx": idx16, "colv": colv, "recipv": recipv, "iot": iot,
            "wt": wt, "bT": bT,
        })

    if spill_rows:
        sidx = np.concatenate(spill_rows)
    else:
        sidx = np.zeros(0, dtype=np.int64)
    return in_maps, row[sidx] if len(sidx) else np.zeros(0, np.int64), \
        col[sidx] if len(sidx) else np.zeros(0, np.int64), recip_full, W, b


# --------------------------------------------------------------------------
# Device program
# --------------------------------------------------------------------------

def _build_nc():
    # full region-overlap analysis so a covering writer resets a tile's
    # dependency set (keeps the gather's sem-wait count within ISA limits)
    os.environ["TILE_EXHAUSTIVE_MEMORY_SHARE_CHECK"] = "1"
    import concourse.bacc as bacc
    import concourse.mybir as mybir
    import concourse.tile as tile
    from concourse import library_config

    dt = mybir.dt
    # 64KB SWDGE descriptor-ring carveout: await_space (decode side) blocks
    # the whole GPSIMD stream when the next gather's descriptors don't fit
    # until prior DMAs complete, so ring capacity == gather pipeline depth
    nc = bacc.Bacc(None, target_bir_lowering=False, debug=False,
                   dynamic_dma_scratch_size=49152, num_swdge_queues=4)

    xlo = nc.dram_tensor("xlo", [HALF, D], dt.bfloat16, kind="ExternalInput").ap()
    xhi = nc.dram_tensor("xhi", [HALF, D], dt.bfloat16, kind="ExternalInput").ap()
    idx = nc.dram_tensor("idx", [128, N_TILES * 8], dt.int16,
                         kind="ExternalInput").ap()
    colv = nc.dram_tensor("colv", [128, N_TILES], dt.bfloat16,
                          kind="ExternalInput").ap()
    recipv = nc.dram_tensor("recipv", [128, N_TILES], dt.bfloat16,
                            kind="ExternalInput").ap()
    iot = nc.dram_tensor("iot", [128, M_WIN], dt.bfloat16,
                         kind="ExternalInput").ap()
    wt = nc.dram_tensor("wt", [D, D], dt.float32, kind="ExternalInput").ap()
    bT = nc.dram_tensor("bT", [128, 1], dt.float32, kind="ExternalInput").ap()
    # transposed output [dout, dst]; host transposes back and slices the pad
    out = nc.dram_tensor("out", [128, TILES_HALF * V_T], dt.float32,
                         kind="ExternalOutput").ap()

    srounds = []
    t0 = 0
    for w in SROUND_W:
        srounds.append((t0, w))
        t0 += w
    assert t0 == TILES_HALF
    # 32-tile compute sub-rounds within each super-round
    rounds = []
    for si, (s0, w) in enumerate(srounds):
        o = 0
        while o < w:
            c = min(32, w - o)
            rounds.append((s0 + o, c, si))
            o += c

    with tile.TileContext(nc) as tc:
        with (
            tc.tile_pool(name="const", bufs=1) as constp,
            tc.tile_pool(name="agg", bufs=1) as aggp,
            tc.tile_pool(name="io", bufs=8) as iop,
            tc.tile_pool(name="sgen", bufs=8) as sgenp,
            tc.tile_pool(name="win", bufs=3) as winp,
            tc.tile_pool(name="psum", bufs=3, space="PSUM") as psp,
            tc.tile_pool(name="psum2", bufs=2, space="PSUM") as ps2p,
        ):
            idx_sb = constp.tile([128, N_TILES * 8], dt.int16)
            colv_sb = constp.tile([128, N_TILES], dt.bfloat16)
            recipv_sb = constp.tile([128, N_TILES], dt.bfloat16)
            iot_sb = constp.tile([128, M_WIN], dt.bfloat16)
            wt_sb = constp.tile([D, D], dt.float32)
            bT_sb = constp.tile([128, 1], dt.float32)

            # all input streams ride mainline SWDGE (qPoolDynamic): the
            # sync/scalar HW-dynamic queues are throttled to ~2.9GB/s per
            # DMA engine (~47GB/s), which paced the whole round pipeline.
            # Whole-tensor loads -> 3-12.5KB per-partition descriptors.
            nc.gpsimd.dma_start(out=idx_sb[:], in_=idx[:, :])
            nc.gpsimd.dma_start(out=colv_sb[:], in_=colv[:, :])
            nc.gpsimd.dma_start(out=recipv_sb[:], in_=recipv[:, :])
            nc.gpsimd.dma_start(out=iot_sb[:], in_=iot[:, :])
            nc.gpsimd.dma_start(out=wt_sb[:], in_=wt[:, :])
            nc.gpsimd.dma_start(out=bT_sb[:], in_=bT[:, :])

            # +M_WIN margin: the last tile of a round spills into the first
            # 16 destinations of the next round's window
            aggT = aggp.tile([128, SHARD + M_WIN], dt.float32)
            nc.vector.memset(aggT[:], 0.0)

            # on-device S generation: S[p, t, c] = recip[p, t] *
            # (iota[c] == col[p, t]), two DVE ops with stride-0 broadcast
            # APs per half.  Emitted TWO ROUNDS AHEAD of consumption so the
            # DVE's in-order stream doesn't chain S-gen(r) behind
            # evict(r-1) (which waits on round r-1's gather completions) —
            # that chain was pacing the whole kernel.
            def emit_sgen(r0, w, _si=None):
                sgs = []
                for h in (0, 1):
                    g0 = h * TILES_HALF + r0
                    sg = sgenp.tile([128, 32 * M_WIN], dt.bfloat16, tag="sg")
                    sg3 = sg[:, :w * M_WIN].rearrange(
                        "p (t c) -> p t c", c=M_WIN)
                    iot3 = iot_sb[:].unsqueeze(1).broadcast_to(
                        (128, w, M_WIN))
                    col3 = colv_sb[:, g0:g0 + w].unsqueeze(2).broadcast_to(
                        (128, w, M_WIN))
                    rec3 = recipv_sb[:, g0:g0 + w].unsqueeze(2).broadcast_to(
                        (128, w, M_WIN))
                    nc.vector.tensor_tensor(
                        out=sg3, in0=iot3, in1=col3,
                        op=mybir.AluOpType.is_equal)
                    nc.vector.tensor_mul(out=sg3, in0=sg3, in1=rec3)
                    sgs.append(sg)
                return sgs

            sg_q = {ri: emit_sgen(*rounds[ri]) for ri in range(2)}

            sr_bufs = {}
            next_sr = 0

            def emit_gathers(si):
                # 4 gather quarters of super-round si: (h, part) with part
                # a=[0,wa) b=[wa,w); quarter j always lands on queue j (and
                # y-buf j mod 4) so same-buffer gathers stay on one FIFO
                # (WAW-safe)
                s0, w = srounds[si]
                wa = (w + 1) // 2
                parts = [(0, 0, wa), (0, wa, w), (1, 0, wa), (1, wa, w)]
                y3s = {}
                for qn, (h, a, bnd) in enumerate(parts):
                    xsrc = xlo if h == 0 else xhi
                    g0 = h * TILES_HALF + s0 + a
                    ntq = bnd - a
                    slots = ntq * 128
                    y_sb = iop.tile([128, 32 * D], dt.bfloat16, tag="y")
                    y3 = y_sb[:, :ntq * D].rearrange("p (t e) -> p t e", e=D)
                    nc.gpsimd.dma_gather(
                        y3, xsrc, idx_sb[:, g0 * 8:(g0 + ntq) * 8],
                        slots, slots, D, elem_step=D, single_packet=False,
                        queue_num=qn)
                    y3s[(h, 0 if a == 0 else 1)] = y3
                sr_bufs[si] = (wa, y3s)

            for ri, (r0, w, si) in enumerate(rounds):
                while next_sr <= si:
                    emit_gathers(next_sr)
                    next_sr += 1

                # S for round ri+2 generated now (2-round DVE lookahead)
                if ri + 2 < len(rounds):
                    sg_q[ri + 2] = emit_sgen(*rounds[ri + 2])
                sgs = sg_q.pop(ri)

                s0 = srounds[si][0]
                wa, y3s = sr_bufs[si]
                ps = psp.tile([128, 32 * M_WIN], dt.float32)
                # both halves accumulate into the same PSUM columns; the
                # start/stop pair of each column range stays adjacent in
                # the PE stream so the accumulation group is well-formed
                for tl in range(w):
                    c0 = tl * M_WIN
                    # split at PSUM bank boundaries
                    cuts = [c0]
                    nb = (c0 // PSUM_BANK + 1) * PSUM_BANK
                    while nb < c0 + M_WIN:
                        cuts.append(nb)
                        nb += PSUM_BANK
                    cuts.append(c0 + M_WIN)
                    tg = (r0 - s0) + tl
                    part = 0 if tg < wa else 1
                    off = tg if tg < wa else tg - wa
                    for h in (0, 1):
                        for a, bnd in zip(cuts[:-1], cuts[1:]):
                            nc.tensor.matmul(
                                out=ps[:, a:bnd],
                                lhsT=y3s[(h, part)][:, off, :],
                                rhs=sgs[h][:, tl * M_WIN + (a - c0):
                                           tl * M_WIN + (bnd - c0)],
                                start=(h == 0), stop=(h == 1),
                            )

                # evict with parity-split strided adds (consecutive tiles'
                # 32-col windows overlap by 16, so even/odd tiles must land
                # in separate vector ops)
                base = r0 * V_T
                ps3 = ps[:, :w * M_WIN].rearrange("p (t c) -> p t c", c=M_WIN)
                n_even = (w + 1) // 2
                n_odd = w // 2
                dst_e = aggT[:, base:base + n_even * M_WIN].rearrange(
                    "p (t c) -> p t c", c=M_WIN)
                nc.vector.tensor_add(out=dst_e, in0=dst_e, in1=ps3[:, 0::2, :])
                if n_odd:
                    dst_o = aggT[:, base + V_T:
                                 base + V_T + n_odd * M_WIN].rearrange(
                        "p (t c) -> p t c", c=M_WIN)
                    nc.vector.tensor_add(
                        out=dst_o, in0=dst_o, in1=ps3[:, 1::2, :])

                # window [base, base+w*16) is complete: final linear
                # (W.T stationary, aggT moving), bias, and store
                ncols = w * V_T
                ps2 = ps2p.tile([128, PSUM_BANK], dt.float32)
                nc.tensor.matmul(
                    out=ps2[:, :ncols],
                    lhsT=wt_sb[:],
                    rhs=aggT[:, base:base + ncols],
                    start=True, stop=True,
                )
                # bias-add on the (otherwise idle) Activation engine keeps
                # the DVE stream free for S-gen + evict
                o_win = winp.tile([128, PSUM_BANK], dt.float32)
                nc.scalar.activation(
                    o_win[:, :ncols], ps2[:, :ncols],
                    mybir.ActivationFunctionType.Identity,
                    bias=bT_sb[:, 0:1], scale=1.0)
                nc.sync.dma_start(
                    out=out[:, base:base + ncols], in_=o_win[:, :ncols])

    # Bacc.compile splits multi-sem waits into EventSemaphore instructions
    # (the walrus ISA allows only one wait per instruction), inserts
    # library reloads, and fills in extended-ISA instruction bytes.
    nc.compile()
    return nc


# --------------------------------------------------------------------------
# Entry point
# --------------------------------------------------------------------------

def kernel(x, edge_index, W, b, _want_profile=False):
    from concourse.bass_utils import run_bass_kernel_spmd

    in_maps, sp_row, sp_col, recip_full, W_np, b_np = _pack_inputs(
        x, edge_index, W, b)

    nc = _build_nc()
    res = run_bass_kernel_spmd(nc, in_maps, list(range(NC)),
                               trace=_want_profile)

    out_full = np.empty((N_NODES, D), dtype=np.float32)
    for k in range(NC):
        out_full[k * SHARD:(k + 1) * SHARD] = res.results[k]["out"][:, :SHARD].T

    # host fixup for spilled edges: out[r] += (x[c] @ W.T) / deg[r]
    if len(sp_row):
        xs = np.asarray(x, dtype=np.float32)[sp_col]
        contrib = xs @ W_np.T
        order = np.argsort(sp_row, kind="stable")
        r_s = sp_row[order]
        c_s = contrib[order] * recip_full[r_s][:, None]
        bounds = np.nonzero(np.diff(r_s))[0] + 1
        starts = np.concatenate(([0], bounds))
        sums = np.add.reduceat(c_s, starts, axis=0)
        out_full[r_s[starts]] += sums

    if _want_profile:
        return out_full, res
    return out_full


if __name__ == "__main__":
    # quick self-exercise with random data
    rng = np.random.default_rng(0)
    x = rng.standard_normal((N_NODES, D), dtype=np.float32)
    ei = rng.integers(0, N_NODES, size=(2, N_EDGES)).astype(np.int64)
    W = rng.standard_normal((D, D), dtype=np.float32) / np.sqrt(D)
    b = rng.standard_normal(D, dtype=np.float32) * 0.01
    out = kernel(x, ei, W, b)
    print("out", out.shape, out.dtype)
